# revision 1
# baseline (speedup 1.0000x reference)
"""Memory-attention kernel for 8x Trainium2 NeuronCores (Bass/Tile).

Problem:  qs [B=8, L=4096, D=1024] f32, memory [M=512, D=1024] f32
          s = einsum("bld,md->bml");  v = softmax(s, axis=2)  (over L)
          r = einsum("bml,bld->bmd")  ->  returns (r, memory)

Sharding: data-parallel over B — one batch per core, memory replicated.
          No collectives needed.

Per-core algorithm (flash-attention-style single pass over L in chunks
of 512, online softmax with per-partition stats):
  - scores S[m,l] = memory @ qs_b.T computed on the PE as
    mh.T@qh + ml.T@qh + mh.T@ql with hi/lo bf16 splits prepared on the
    host (error ~2^-18 per product -> fp32-class score accuracy; 3 bf16
    matmuls run 4/3x faster than one fp32 matmul on TRN2's PE).
  - row max via DVE reduce (negated), exp+row-sum fused on the ACT
    engine (accum_out), online rescale of the running output in the
    flash style (scalar_tensor_tensor: acc*corr + chunk_psum).
  - V^T obtained with PE transpose-mode (128x128 blocks).
  - r-chunk matmul V^T.T @ qs in float32r (TF32-like PE fast path,
    1 cyc/row vs fp32's 4; adds ~1e-3 abs error on a ~5.2-scale output).
Inputs are shipped pre-transposed/pre-split/pre-tiled from the host so
every DMA lands as 128 partitions x 16KB contiguous (peak HBM BW).
"""

from contextlib import ExitStack, nullcontext

import numpy as np
import ml_dtypes

import concourse.bass as bass
import concourse.tile as tile
import concourse.mybir as mybir
from concourse import bacc
from concourse.alu_op_type import AluOpType
from concourse.bass_utils import run_bass_kernel_spmd
from concourse.masks import make_identity

F32 = mybir.dt.float32
F32R = mybir.dt.float32r
BF16 = mybir.dt.bfloat16
AF = mybir.ActivationFunctionType
AX = mybir.AxisListType

B, L, D, M = 8, 4096, 1024, 512
CHUNK = 512
DT = D // 128    # 8 d-blocks
MT = M // 128    # 4 m-tiles
NCH = L // CHUNK
LBN = CHUNK // 128
ND2 = D // 512

_CACHE = {}


def _build(loop_n=1, bufs=3):
    nc = bacc.Bacc("TRN2", target_bir_lowering=False, debug=False,
                   num_devices=B)

    qsT_hi_ap = nc.dram_tensor("qsT_hi", [NCH, 128, DT, CHUNK], BF16,
                               kind="ExternalInput").ap()
    qsT_lo_ap = nc.dram_tensor("qsT_lo", [NCH, 128, DT, CHUNK], BF16,
                               kind="ExternalInput").ap()
    memT_hi_ap = nc.dram_tensor("memT_hi", [D, M], BF16,
                                kind="ExternalInput").ap()
    memT_lo_ap = nc.dram_tensor("memT_lo", [D, M], BF16,
                                kind="ExternalInput").ap()
    qs_ap = nc.dram_tensor("qs", [NCH, 128, LBN, D], F32R,
                           kind="ExternalInput").ap()
    r_ap = nc.dram_tensor("r", [M, D], F32, kind="ExternalOutput").ap()

    with tile.TileContext(nc) as tc, ExitStack() as ctx:
        const_p = ctx.enter_context(tc.tile_pool(name="const", bufs=1))
        acc_p = ctx.enter_context(tc.tile_pool(name="acc", bufs=1))
        qsT_p = ctx.enter_context(tc.tile_pool(name="qsT", bufs=bufs))
        qs_p = ctx.enter_context(tc.tile_pool(name="qs", bufs=bufs))
        v_p = ctx.enter_context(tc.tile_pool(name="v", bufs=2 * MT))
        vt_p = ctx.enter_context(tc.tile_pool(name="vt", bufs=2 * LBN))
        st_p = ctx.enter_context(tc.tile_pool(name="st", bufs=4 * MT))
        out_p = ctx.enter_context(tc.tile_pool(name="out", bufs=2))
        ps_s = ctx.enter_context(tc.tile_pool(name="ps_s", bufs=2, space="PSUM"))
        ps_vt = ctx.enter_context(tc.tile_pool(name="ps_vt", bufs=2, space="PSUM"))
        ps_r = ctx.enter_context(tc.tile_pool(name="ps_r", bufs=2, space="PSUM"))

        ident_f = const_p.tile([128, 128], F32, tag="ident_f")
        make_identity(nc, ident_f[:])
        ident = const_p.tile([128, 128], F32R, tag="ident")
        nc.vector.tensor_copy(ident[:], ident_f[:])

        memT_hi = const_p.tile([128, DT, M], BF16, tag="memT_hi")
        nc.sync.dma_start(
            memT_hi[:], memT_hi_ap.rearrange("(k p) m -> p k m", p=128))
        memT_lo = const_p.tile([128, DT, M], BF16, tag="memT_lo")
        nc.sync.dma_start(
            memT_lo[:], memT_lo_ap.rearrange("(k p) m -> p k m", p=128))

        r_acc = [acc_p.tile([128, D], F32, tag=f"racc{mt}", name=f"racc{mt}")
                 for mt in range(MT)]

        # measurement-only repeat loop (loop_n=1 in production)
        loop_cm = tc.For_i(0, loop_n, 1) if loop_n > 1 else nullcontext()
        ctx.enter_context(loop_cm)
        nrun = [None] * MT   # negative running max, [128,1] per m-tile
        rsum = [None] * MT   # running sum of exp

        for c in range(NCH):
            qsT_hi_c = qsT_p.tile([128, DT, CHUNK], BF16, tag="qsT_hi")
            nc.sync.dma_start(qsT_hi_c[:], qsT_hi_ap[c])
            qsT_lo_c = qsT_p.tile([128, DT, CHUNK], BF16, tag="qsT_lo")
            nc.sync.dma_start(qsT_lo_c[:], qsT_lo_ap[c])
            qs_c = qs_p.tile([128, LBN, D], F32R, tag="qs")
            nc.sync.dma_start(qs_c[:], qs_ap[c])

            v_sb = []
            corr = []
            for mt in range(MT):
                # scores: psum [128 m, CHUNK l], 3 hi/lo terms x 8 k-blocks
                s_ps = ps_s.tile([128, CHUNK], F32, tag="s")
                msl = slice(mt * 128, (mt + 1) * 128)
                groups = [(memT_hi, qsT_hi_c), (memT_lo, qsT_hi_c),
                          (memT_hi, qsT_lo_c)]
                n_mm = len(groups) * DT
                i = 0
                for m_op, q_op in groups:
                    for k in range(DT):
                        nc.tensor.matmul(
                            s_ps[:], m_op[:, k, msl], q_op[:, k, :],
                            start=(i == 0), stop=(i == n_mm - 1))
                        i += 1

                # online softmax stats
                nmax_c = st_p.tile([128, 1], F32, tag="nmax")
                nc.vector.reduce_max(nmax_c[:], s_ps[:], axis=AX.X, negate=True)
                if c == 0:
                    new_nrun = nmax_c
                    cr = None
                else:
                    new_nrun = st_p.tile([128, 1], F32, tag="nmax")
                    nc.vector.tensor_tensor(
                        new_nrun[:], nrun[mt][:], nmax_c[:], op=AluOpType.min)
                    delta = st_p.tile([128, 1], F32, tag="delta")
                    nc.vector.tensor_sub(delta[:], new_nrun[:], nrun[mt][:])
                    cr = st_p.tile([128, 1], F32, tag="corr")
                    nc.scalar.activation(cr[:], delta[:], AF.Exp)
                corr.append(cr)
                nrun[mt] = new_nrun

                # exp with per-partition bias (-max), fused row-sum
                v_mt = v_p.tile([128, CHUNK], F32R, tag="v")
                csum = st_p.tile([128, 1], F32, tag="csum")
                nc.scalar.activation(
                    v_mt[:], s_ps[:], AF.Exp,
                    bias=new_nrun[:], accum_out=csum[:])
                v_sb.append(v_mt)

                rs = st_p.tile([128, 1], F32, tag="rsum")
                if c == 0:
                    nc.vector.tensor_copy(rs[:], csum[:])
                else:
                    nc.vector.scalar_tensor_tensor(
                        rs[:], rsum[mt][:], cr[:], csum[:],
                        op0=AluOpType.mult, op1=AluOpType.add)
                rsum[mt] = rs

            # V [m,l] -> V^T [l,m] via PE transpose-mode
            vt_sb = []
            for lb in range(LBN):
                vt_ps = ps_vt.tile([128, M], F32R, tag="vt")
                for mt in range(MT):
                    nc.tensor.transpose(
                        vt_ps[:, mt * 128:(mt + 1) * 128],
                        v_sb[mt][:, lb * 128:(lb + 1) * 128],
                        ident[:])
                vt_t = vt_p.tile([128, M], F32R, tag="vt_sb")
                nc.scalar.copy(vt_t[:], vt_ps[:])
                vt_sb.append(vt_t)

            # r-chunk matmuls (f32r) + flash accumulate
            for mt in range(MT):
                for dh in range(ND2):
                    r_ps = ps_r.tile([128, 512], F32, tag="r")
                    for lb in range(LBN):
                        nc.tensor.matmul(
                            r_ps[:],
                            vt_sb[lb][:, mt * 128:(mt + 1) * 128],
                            qs_c[:, lb, dh * 512:(dh + 1) * 512],
                            start=(lb == 0), stop=(lb == LBN - 1))
                    dst = r_acc[mt][:, dh * 512:(dh + 1) * 512]
                    if c == 0:
                        nc.scalar.copy(dst, r_ps[:])
                    else:
                        nc.vector.scalar_tensor_tensor(
                            dst, dst, corr[mt][:], r_ps[:],
                            op0=AluOpType.mult, op1=AluOpType.add)

        # finalize: r = r_acc / rsum
        for mt in range(MT):
            rinv = st_p.tile([128, 1], F32, tag="rinv")
            nc.vector.reciprocal(rinv[:], rsum[mt][:])
            o_sb = out_p.tile([128, D], F32, tag="o")
            nc.vector.tensor_scalar_mul(o_sb[:], r_acc[mt][:], rinv[:])
            nc.sync.dma_start(
                r_ap.rearrange("(t p) d -> t p d", p=128)[mt], o_sb[:])

    nc.compile()
    return nc


def get_nc(loop_n=1):
    if loop_n not in _CACHE:
        _CACHE[loop_n] = _build(loop_n=loop_n)
    return _CACHE[loop_n]


def _pack_qsT(qsT):
    # [D, L] -> [NCH, 128, DT, CHUNK]: per chunk, 16KB contiguous/partition
    return np.ascontiguousarray(
        qsT.reshape(DT, 128, NCH, CHUNK).transpose(2, 1, 0, 3))


def _pack_qs(qs_b):
    # [L, D] -> [NCH, 128, LBN, D]
    return np.ascontiguousarray(
        qs_b.reshape(NCH, LBN, 128, D).transpose(0, 2, 1, 3))


def core_inputs(qs_b, memory):
    qs_b = np.ascontiguousarray(qs_b, dtype=np.float32)
    qsT = np.ascontiguousarray(qs_b.T)
    memT = np.ascontiguousarray(memory.T.astype(np.float32))
    qhi = qsT.astype(ml_dtypes.bfloat16)
    qlo = (qsT - qhi.astype(np.float32)).astype(ml_dtypes.bfloat16)
    mhi = memT.astype(ml_dtypes.bfloat16)
    mlo = (memT - mhi.astype(np.float32)).astype(ml_dtypes.bfloat16)
    return {
        "qs": _pack_qs(qs_b),
        "qsT_hi": _pack_qsT(qhi),
        "qsT_lo": _pack_qsT(qlo),
        "memT_hi": mhi,
        "memT_lo": mlo,
    }


def kernel(qs, memory):
    """Full inputs in, full outputs out. Shards batch over 8 cores."""
    qs = np.asarray(qs, dtype=np.float32)
    memory = np.asarray(memory, dtype=np.float32)
    assert qs.shape == (B, L, D) and memory.shape == (M, D)

    nc = get_nc()
    in_maps = [core_inputs(qs[b], memory) for b in range(B)]
    res = run_bass_kernel_spmd(nc, in_maps, list(range(B)))
    r = np.stack([res.results[b]["r"] for b in range(B)], axis=0)
    return (r, memory)
